# revision 3
# baseline (speedup 1.0000x reference)
"""KMeans VQ kernel v3 — bf16 hi/lo GEMM, compact argmin output,
vsq split between PE (wide rank-3 matmul) and ACT (batched PSUM copy).

Per chunk of 4 subtiles (PSUM [P, 4, K] f32):
  vsq init: chunk%4==0 -> PE wide rank-3 matmul (ones3 @ vsq3_x4, 1024 cols)
            else       -> ACT batched copy vsqr4 [P,4,K] f32 SBUF->PSUM
  PE: per subtile 3 bf16 matmuls (hi*whi, hi*wlo, lo*whi) accumulate
  DVE: batched tensor_reduce min -> mrow_all[:, 4 cols]
  ACT: per subtile Sign(mrow - m_s) -> ind bf16 {0 @ argmin, -1}
  DVE: per subtile stt (ind+1)*iota_sc, dummy broadcast out,
       accum SUM -> idx_all col = (255-k*)/256
Host: idx = 255-256*accum, s = sqrt(max(mrow+xsq,0)), scatter one-hot rows.
X is host-pre-transposed: loads are plain contiguous DMA (no xbar).
"""

import os
import sys

import numpy as np

sys.path.insert(0, "/opt/trn_rl_repo")

N = 500000
D = 128
K = 256
N_CORES = 8
P = 128
NPC = 62976
N_PAD = N_CORES * NPC
NSUB = NPC // P  # 492
BLOCK_SUBS = [32] * 15 + [12]
PE_VSQ_EVERY = int(os.environ.get("KMEANS_PE_VSQ_EVERY", "1"))  # all chunks: vsq on PE

_nc_cache = {}


def _build():
    from contextlib import ExitStack

    import concourse.bass as bass
    import concourse.tile as tile
    import concourse.tile_sem_assignment as tsa
    from concourse import mybir

    tsa.NUM_HWDGE_SEMS = 1

    f32 = mybir.dt.float32
    bf16 = mybir.dt.bfloat16
    Alu = mybir.AluOpType
    Act = mybir.ActivationFunctionType
    Ax = mybir.AxisListType

    nc = bass.Bass(trn_type="TRN2")
    xhiT_d = nc.dram_tensor("xhiT", [D, NPC], bf16, kind="ExternalInput")
    xloT_d = nc.dram_tensor("xloT", [D, NPC], bf16, kind="ExternalInput")
    whi_d = nc.dram_tensor("whi", [D, K], bf16, kind="ExternalInput")
    wlo_d = nc.dram_tensor("wlo", [D, K], bf16, kind="ExternalInput")
    ones3_d = nc.dram_tensor("ones3", [3, P], bf16, kind="ExternalInput")
    vsq3x4_d = nc.dram_tensor("vsq3x4", [3, 4 * K], bf16, kind="ExternalInput")
    vsqr4_d = nc.dram_tensor("vsqr4", [P, 4 * K], f32, kind="ExternalInput")
    iota_d = nc.dram_tensor("iota_sc", [P, K], bf16, kind="ExternalInput")
    mrow_out_d = nc.dram_tensor("mrow_out", [P, NSUB], f32, kind="ExternalOutput")
    idx_out_d = nc.dram_tensor("idx_out", [P, NSUB], bf16, kind="ExternalOutput")

    def _split_multiwait():
        cnt = 0
        for fn in nc.m.functions:
            for bb in fn.blocks:
                insts = list(bb.instructions)
                out = []
                changed = False
                for ins in insts:
                    si = getattr(ins, "sync_info", None)
                    waits = list(si.on_wait) if (si and si.on_wait) else []
                    if len(waits) > 1:
                        changed = True
                        for wt in waits[:-1]:
                            cnt += 1
                            dr = mybir.InstDrain(name=f"antw-{cnt}", ins=[], outs=[])
                            dr.engine = ins.engine
                            dr.sync_info = mybir.SyncInfo(on_wait=[wt], on_update=[])
                            out.append(dr)
                        ins.sync_info = mybir.SyncInfo(
                            on_wait=[waits[-1]], on_update=list(si.on_update)
                        )
                    out.append(ins)
                if changed:
                    bb.instructions = out
        return cnt

    with tile.TileContext(nc) as tc, ExitStack() as ctx:
        singles = ctx.enter_context(tc.tile_pool(name="singles", bufs=1))
        whi_sb = singles.tile([D, K], bf16)
        nc.sync.dma_start(out=whi_sb, in_=whi_d[:, :])
        wlo_sb = singles.tile([D, K], bf16)
        nc.sync.dma_start(out=wlo_sb, in_=wlo_d[:, :])
        ones3_sb = singles.tile([3, P], bf16)
        nc.sync.dma_start(out=ones3_sb, in_=ones3_d[:, :])
        vsq3x4_sb = singles.tile([3, 4 * K], bf16)
        nc.sync.dma_start(out=vsq3x4_sb, in_=vsq3x4_d[:, :])
        vsqr4_sb = singles.tile([P, 4, K], f32)
        nc.sync.dma_start(out=vsqr4_sb, in_=vsqr4_d[:, :].rearrange("p (g k) -> p g k", g=4))
        iota_sb = singles.tile([P, K], bf16)
        nc.sync.dma_start(out=iota_sb, in_=iota_d[:, :])
        mrow_all = singles.tile([P, NSUB], f32)
        idx_all = singles.tile([P, NSUB], bf16)

        xhip = ctx.enter_context(tc.tile_pool(name="xhiT", bufs=2))
        xlop = ctx.enter_context(tc.tile_pool(name="xloT", bufs=2))
        mps = ctx.enter_context(tc.tile_pool(name="mps", bufs=4, space="PSUM"))
        indp = ctx.enter_context(tc.tile_pool(name="indp", bufs=8))
        dmyp = ctx.enter_context(tc.tile_pool(name="dmyp", bufs=4))

        col0 = 0
        sub0 = 0
        chunk_id = 0
        for nsub in BLOCK_SUBS:
            bcols = nsub * P
            xhi_blk = xhip.tile([D, bcols], bf16, tag="xhib")
            nc.sync.dma_start(out=xhi_blk, in_=xhiT_d[:, col0 : col0 + bcols])
            xlo_blk = xlop.tile([D, bcols], bf16, tag="xlob")
            nc.sync.dma_start(out=xlo_blk, in_=xloT_d[:, col0 : col0 + bcols])

            for q in range(0, nsub, 4):
                m_pc = mps.tile([P, 4, K], f32)
                on_pe = (chunk_id % PE_VSQ_EVERY) == 0
                if on_pe:
                    # wide rank-3: one matmul per PSUM bank (512 cols max)
                    for h in range(2):
                        nc.tensor.matmul(
                            m_pc[:, 2 * h : 2 * h + 2, :],
                            lhsT=ones3_sb,
                            rhs=vsq3x4_sb[:, 2 * h * K : (2 * h + 2) * K],
                            start=True,
                            stop=False,
                            skip_group_check=True,
                        )
                else:
                    nc.scalar.copy(out=m_pc[:, :, :], in_=vsqr4_sb)
                for g in range(4):
                    j = q + g
                    sl = m_pc[:, g, :]
                    hi_sl = xhi_blk[:, j * P : (j + 1) * P]
                    lo_sl = xlo_blk[:, j * P : (j + 1) * P]
                    nc.tensor.matmul(
                        sl, lhsT=hi_sl, rhs=whi_sb,
                        start=False, stop=False, skip_group_check=True,
                    )
                    nc.tensor.matmul(
                        sl, lhsT=hi_sl, rhs=wlo_sb,
                        start=False, stop=False, skip_group_check=True,
                    )
                    nc.tensor.matmul(
                        sl, lhsT=lo_sl, rhs=whi_sb,
                        start=False, stop=True, skip_group_check=True,
                    )

                mrow_c = mrow_all[:, sub0 + q : sub0 + q + 4]
                nc.vector.tensor_reduce(
                    out=mrow_c, in_=m_pc, axis=Ax.X, op=Alu.min
                )

                for g in range(4):
                    j = q + g
                    ind = indp.tile([P, K], bf16, tag="ind")
                    nc.scalar.activation(
                        ind,
                        m_pc[:, g, :],
                        Act.Sign,
                        bias=mrow_all[:, sub0 + j : sub0 + j + 1],
                        scale=-1.0,
                    )
                    dummy = dmyp.tile([P, 1], bf16, tag="dm")
                    nc.vector.scalar_tensor_tensor(
                        out=dummy.broadcast_to((P, K)),
                        in0=ind,
                        scalar=1.0,
                        in1=iota_sb,
                        op0=Alu.add,
                        op1=Alu.mult,
                        accum_out=idx_all[:, sub0 + j : sub0 + j + 1],
                    )
                chunk_id += 1

            col0 += bcols
            sub0 += nsub

        nc.scalar.dma_start(out=mrow_out_d[:, :], in_=mrow_all)
        nc.scalar.dma_start(out=idx_out_d[:, :], in_=idx_all)

    _split_multiwait()
    from concourse.library_overlay import lower_extended_insts

    lower_extended_insts(nc)
    return nc


def _host_prep(X: np.ndarray, V: np.ndarray):
    import ml_dtypes

    bf = ml_dtypes.bfloat16
    V = np.asarray(V, dtype=np.float32)
    wt = np.ascontiguousarray((-2.0 * V).T)  # [D, K] f32

    vsq = np.sum(V * V, axis=1, dtype=np.float32)
    v1 = vsq.astype(bf)
    r = vsq - v1.astype(np.float32)
    v2 = r.astype(bf)
    v3 = (r - v2.astype(np.float32)).astype(bf)
    vsq3 = np.stack([v1, v2, v3])  # [3, K]
    vsq3x4 = np.ascontiguousarray(np.tile(vsq3, (1, 4)))  # [3, 4K]
    ones3 = np.ones((3, P), dtype=bf)
    vsqr4 = np.ascontiguousarray(
        np.broadcast_to(np.tile(vsq, 4)[None, :], (P, 4 * K))
    ).astype(np.float32)
    iota_sc = np.ascontiguousarray(
        np.broadcast_to(
            ((255.0 - np.arange(K, dtype=np.float32)) / 256.0).astype(bf)[None, :],
            (P, K),
        )
    )

    xp = np.zeros((N_PAD, D), dtype=np.float32)
    xp[:N] = X
    xsq = np.einsum("nd,nd->n", xp, xp).astype(np.float32)

    whi = wt.astype(bf)
    wlo = (wt - whi.astype(np.float32)).astype(bf)
    common = dict(
        whi=whi, wlo=wlo, ones3=ones3, vsq3x4=vsq3x4, vsqr4=vsqr4, iota_sc=iota_sc
    )
    per_core = []
    for c in range(N_CORES):
        sl = xp[c * NPC : (c + 1) * NPC]
        hi = sl.astype(bf)
        lo = (sl - hi.astype(np.float32)).astype(bf)
        per_core.append(
            dict(
                xhiT=np.ascontiguousarray(hi.T),
                xloT=np.ascontiguousarray(lo.T),
                **common,
            )
        )
    return per_core, xsq


def kernel(X: np.ndarray, V: np.ndarray) -> np.ndarray:
    from concourse.bass_utils import run_bass_kernel_spmd

    X = np.asarray(X, dtype=np.float32)
    in_maps, xsq = _host_prep(X, V)

    if "h" not in _nc_cache:
        _nc_cache["h"] = _build()
    nc = _nc_cache["h"]

    trace = bool(int(os.environ.get("KMEANS_TRACE", "0")))
    res = run_bass_kernel_spmd(
        nc, in_maps, core_ids=list(range(N_CORES)), trace=trace
    )
    if trace and res.exec_time_ns is not None:
        kernel.last_exec_time_ns = res.exec_time_ns
        kernel.last_mean_exec_time_ns = res.mean_exec_time_ns
        kernel.last_trace = res.instructions_and_trace

    mrow = np.concatenate([r["mrow_out"].T.reshape(-1) for r in res.results])[:N]
    idxf = np.concatenate(
        [r["idx_out"].astype(np.float32).T.reshape(-1) for r in res.results]
    )[:N]
    idx = np.rint(255.0 - 256.0 * idxf).astype(np.int64)
    np.clip(idx, 0, K - 1, out=idx)
    s = np.sqrt(np.maximum(mrow + xsq[:N], 0.0)).astype(np.float32)
    out = np.zeros((N, K), dtype=np.float32)
    out[np.arange(N), idx] = s
    return out


kernel.last_exec_time_ns = None
kernel.last_mean_exec_time_ns = None
kernel.last_trace = None


# revision 4
# speedup vs baseline: 1.0052x; 1.0052x over previous
"""KMeans VQ kernel v3 — bf16 hi/lo GEMM, compact argmin output,
vsq split between PE (wide rank-3 matmul) and ACT (batched PSUM copy).

Per chunk of 4 subtiles (PSUM [P, 4, K] f32):
  vsq init: chunk%4==0 -> PE wide rank-3 matmul (ones3 @ vsq3_x4, 1024 cols)
            else       -> ACT batched copy vsqr4 [P,4,K] f32 SBUF->PSUM
  PE: per subtile 3 bf16 matmuls (hi*whi, hi*wlo, lo*whi) accumulate
  DVE: batched tensor_reduce min -> mrow_all[:, 4 cols]
  ACT: per subtile Sign(mrow - m_s) -> ind bf16 {0 @ argmin, -1}
  DVE: per subtile stt (ind+1)*iota_sc, dummy broadcast out,
       accum SUM -> idx_all col = (255-k*)/256
Host: idx = 255-256*accum, s = sqrt(max(mrow+xsq,0)), scatter one-hot rows.
X is host-pre-transposed: loads are plain contiguous DMA (no xbar).
"""

import os
import sys

import numpy as np

sys.path.insert(0, "/opt/trn_rl_repo")

N = 500000
D = 128
K = 256
N_CORES = 8
P = 128
NPC = 62976
N_PAD = N_CORES * NPC
NSUB = NPC // P  # 492
BLOCK_SUBS = [32] * 15 + [12]
PE_VSQ_EVERY = int(os.environ.get("KMEANS_PE_VSQ_EVERY", "1"))  # all chunks: vsq on PE

_nc_cache = {}


def _build():
    from contextlib import ExitStack

    import concourse.bass as bass
    import concourse.tile as tile
    import concourse.tile_sem_assignment as tsa
    from concourse import mybir

    tsa.NUM_HWDGE_SEMS = 1

    f32 = mybir.dt.float32
    bf16 = mybir.dt.bfloat16
    Alu = mybir.AluOpType
    Act = mybir.ActivationFunctionType
    Ax = mybir.AxisListType

    nc = bass.Bass(trn_type="TRN2")
    xhiT_d = nc.dram_tensor("xhiT", [D, NPC], bf16, kind="ExternalInput")
    xloT_d = nc.dram_tensor("xloT", [D, NPC], bf16, kind="ExternalInput")
    whi_d = nc.dram_tensor("whi", [D, K], bf16, kind="ExternalInput")
    wlo_d = nc.dram_tensor("wlo", [D, K], bf16, kind="ExternalInput")
    ones3_d = nc.dram_tensor("ones3", [3, P], bf16, kind="ExternalInput")
    vsq3x4_d = nc.dram_tensor("vsq3x4", [3, 4 * K], bf16, kind="ExternalInput")
    vsqr4_d = nc.dram_tensor("vsqr4", [P, 4 * K], f32, kind="ExternalInput")
    iota_d = nc.dram_tensor("iota_sc", [P, K], bf16, kind="ExternalInput")
    mrow_out_d = nc.dram_tensor("mrow_out", [P, NSUB], f32, kind="ExternalOutput")
    idx_out_d = nc.dram_tensor("idx_out", [P, NSUB], bf16, kind="ExternalOutput")

    def _split_multiwait():
        cnt = 0
        for fn in nc.m.functions:
            for bb in fn.blocks:
                insts = list(bb.instructions)
                out = []
                changed = False
                for ins in insts:
                    si = getattr(ins, "sync_info", None)
                    waits = list(si.on_wait) if (si and si.on_wait) else []
                    if len(waits) > 1:
                        changed = True
                        for wt in waits[:-1]:
                            cnt += 1
                            dr = mybir.InstDrain(name=f"antw-{cnt}", ins=[], outs=[])
                            dr.engine = ins.engine
                            dr.sync_info = mybir.SyncInfo(on_wait=[wt], on_update=[])
                            out.append(dr)
                        ins.sync_info = mybir.SyncInfo(
                            on_wait=[waits[-1]], on_update=list(si.on_update)
                        )
                    out.append(ins)
                if changed:
                    bb.instructions = out
        return cnt

    with tile.TileContext(nc) as tc, ExitStack() as ctx:
        singles = ctx.enter_context(tc.tile_pool(name="singles", bufs=1))
        whi_sb = singles.tile([D, K], bf16)
        nc.sync.dma_start(out=whi_sb, in_=whi_d[:, :])
        wlo_sb = singles.tile([D, K], bf16)
        nc.sync.dma_start(out=wlo_sb, in_=wlo_d[:, :])
        ones3_sb = singles.tile([3, P], bf16)
        nc.sync.dma_start(out=ones3_sb, in_=ones3_d[:, :])
        vsq3x4_sb = singles.tile([3, 4 * K], bf16)
        nc.sync.dma_start(out=vsq3x4_sb, in_=vsq3x4_d[:, :])
        vsqr4_sb = singles.tile([P, 4, K], f32)
        nc.sync.dma_start(out=vsqr4_sb, in_=vsqr4_d[:, :].rearrange("p (g k) -> p g k", g=4))
        iota_sb = singles.tile([P, K], bf16)
        nc.sync.dma_start(out=iota_sb, in_=iota_d[:, :])
        mrow_all = singles.tile([P, NSUB], f32)
        idx_all = singles.tile([P, NSUB], bf16)

        xhip = ctx.enter_context(tc.tile_pool(name="xhiT", bufs=2))
        xlop = ctx.enter_context(tc.tile_pool(name="xloT", bufs=2))
        mps = ctx.enter_context(tc.tile_pool(name="mps", bufs=4, space="PSUM"))
        indp = ctx.enter_context(tc.tile_pool(name="indp", bufs=8))
        dmyp = ctx.enter_context(tc.tile_pool(name="dmyp", bufs=4))

        col0 = 0
        sub0 = 0
        chunk_id = 0
        pending = []

        def _extract(pc, base):
            for g in range(4):
                ind = indp.tile([P, K], bf16, tag="ind")
                nc.scalar.activation(
                    ind,
                    pc[:, g, :],
                    Act.Sign,
                    bias=mrow_all[:, base + g : base + g + 1],
                    scale=-1.0,
                )
                dummy = dmyp.tile([P, 1], bf16, tag="dm")
                nc.vector.scalar_tensor_tensor(
                    out=dummy.broadcast_to((P, K)),
                    in0=ind,
                    scalar=1.0,
                    in1=iota_sb,
                    op0=Alu.add,
                    op1=Alu.mult,
                    accum_out=idx_all[:, base + g : base + g + 1],
                )
        for nsub in BLOCK_SUBS:
            bcols = nsub * P
            xhi_blk = xhip.tile([D, bcols], bf16, tag="xhib")
            nc.sync.dma_start(out=xhi_blk, in_=xhiT_d[:, col0 : col0 + bcols])
            xlo_blk = xlop.tile([D, bcols], bf16, tag="xlob")
            nc.sync.dma_start(out=xlo_blk, in_=xloT_d[:, col0 : col0 + bcols])

            for q in range(0, nsub, 4):
                m_pc = mps.tile([P, 4, K], f32)
                on_pe = (chunk_id % PE_VSQ_EVERY) == 0
                if on_pe:
                    # wide rank-3: one matmul per PSUM bank (512 cols max)
                    for h in range(2):
                        nc.tensor.matmul(
                            m_pc[:, 2 * h : 2 * h + 2, :],
                            lhsT=ones3_sb,
                            rhs=vsq3x4_sb[:, 2 * h * K : (2 * h + 2) * K],
                            start=True,
                            stop=False,
                            skip_group_check=True,
                        )
                else:
                    nc.scalar.copy(out=m_pc[:, :, :], in_=vsqr4_sb)
                for g in range(4):
                    j = q + g
                    sl = m_pc[:, g, :]
                    hi_sl = xhi_blk[:, j * P : (j + 1) * P]
                    lo_sl = xlo_blk[:, j * P : (j + 1) * P]
                    nc.tensor.matmul(
                        sl, lhsT=hi_sl, rhs=whi_sb,
                        start=False, stop=False, skip_group_check=True,
                    )
                    nc.tensor.matmul(
                        sl, lhsT=hi_sl, rhs=wlo_sb,
                        start=False, stop=False, skip_group_check=True,
                    )
                    nc.tensor.matmul(
                        sl, lhsT=lo_sl, rhs=whi_sb,
                        start=False, stop=True, skip_group_check=True,
                    )

                mrow_c = mrow_all[:, sub0 + q : sub0 + q + 4]
                nc.vector.tensor_reduce(
                    out=mrow_c, in_=m_pc, axis=Ax.X, op=Alu.min
                )

                # software-pipeline: extract indices for the PREVIOUS chunk so
                # DVE/ACT never wait on deps produced this iteration
                pending.append((m_pc, sub0 + q))
                if len(pending) > 1:
                    _extract(*pending.pop(0))
                chunk_id += 1

            col0 += bcols
            sub0 += nsub

        while pending:
            _extract(*pending.pop(0))

        nc.scalar.dma_start(out=mrow_out_d[:, :], in_=mrow_all)
        nc.scalar.dma_start(out=idx_out_d[:, :], in_=idx_all)

    _split_multiwait()
    from concourse.library_overlay import lower_extended_insts

    lower_extended_insts(nc)
    return nc


def _host_prep(X: np.ndarray, V: np.ndarray):
    import ml_dtypes

    bf = ml_dtypes.bfloat16
    V = np.asarray(V, dtype=np.float32)
    wt = np.ascontiguousarray((-2.0 * V).T)  # [D, K] f32

    vsq = np.sum(V * V, axis=1, dtype=np.float32)
    v1 = vsq.astype(bf)
    r = vsq - v1.astype(np.float32)
    v2 = r.astype(bf)
    v3 = (r - v2.astype(np.float32)).astype(bf)
    vsq3 = np.stack([v1, v2, v3])  # [3, K]
    vsq3x4 = np.ascontiguousarray(np.tile(vsq3, (1, 4)))  # [3, 4K]
    ones3 = np.ones((3, P), dtype=bf)
    vsqr4 = np.ascontiguousarray(
        np.broadcast_to(np.tile(vsq, 4)[None, :], (P, 4 * K))
    ).astype(np.float32)
    iota_sc = np.ascontiguousarray(
        np.broadcast_to(
            ((255.0 - np.arange(K, dtype=np.float32)) / 256.0).astype(bf)[None, :],
            (P, K),
        )
    )

    xp = np.zeros((N_PAD, D), dtype=np.float32)
    xp[:N] = X
    xsq = np.einsum("nd,nd->n", xp, xp).astype(np.float32)

    whi = wt.astype(bf)
    wlo = (wt - whi.astype(np.float32)).astype(bf)
    common = dict(
        whi=whi, wlo=wlo, ones3=ones3, vsq3x4=vsq3x4, vsqr4=vsqr4, iota_sc=iota_sc
    )
    per_core = []
    for c in range(N_CORES):
        sl = xp[c * NPC : (c + 1) * NPC]
        hi = sl.astype(bf)
        lo = (sl - hi.astype(np.float32)).astype(bf)
        per_core.append(
            dict(
                xhiT=np.ascontiguousarray(hi.T),
                xloT=np.ascontiguousarray(lo.T),
                **common,
            )
        )
    return per_core, xsq


def kernel(X: np.ndarray, V: np.ndarray) -> np.ndarray:
    from concourse.bass_utils import run_bass_kernel_spmd

    X = np.asarray(X, dtype=np.float32)
    in_maps, xsq = _host_prep(X, V)

    if "h" not in _nc_cache:
        _nc_cache["h"] = _build()
    nc = _nc_cache["h"]

    trace = bool(int(os.environ.get("KMEANS_TRACE", "0")))
    res = run_bass_kernel_spmd(
        nc, in_maps, core_ids=list(range(N_CORES)), trace=trace
    )
    if trace and res.exec_time_ns is not None:
        kernel.last_exec_time_ns = res.exec_time_ns
        kernel.last_mean_exec_time_ns = res.mean_exec_time_ns
        kernel.last_trace = res.instructions_and_trace

    mrow = np.concatenate([r["mrow_out"].T.reshape(-1) for r in res.results])[:N]
    idxf = np.concatenate(
        [r["idx_out"].astype(np.float32).T.reshape(-1) for r in res.results]
    )[:N]
    idx = np.rint(255.0 - 256.0 * idxf).astype(np.int64)
    np.clip(idx, 0, K - 1, out=idx)
    s = np.sqrt(np.maximum(mrow + xsq[:N], 0.0)).astype(np.float32)
    out = np.zeros((N, K), dtype=np.float32)
    out[np.arange(N), idx] = s
    return out


kernel.last_exec_time_ns = None
kernel.last_mean_exec_time_ns = None
kernel.last_trace = None


# revision 5
# speedup vs baseline: 1.1823x; 1.1762x over previous
"""KMeans VQ kernel v3 — bf16 hi/lo GEMM, compact argmin output,
vsq split between PE (wide rank-3 matmul) and ACT (batched PSUM copy).

Per chunk of 4 subtiles (PSUM [P, 4, K] f32):
  vsq init: chunk%4==0 -> PE wide rank-3 matmul (ones3 @ vsq3_x4, 1024 cols)
            else       -> ACT batched copy vsqr4 [P,4,K] f32 SBUF->PSUM
  PE: per subtile 3 bf16 matmuls (hi*whi, hi*wlo, lo*whi) accumulate
  DVE: batched tensor_reduce min -> mrow_all[:, 4 cols]
  ACT: per subtile Sign(mrow - m_s) -> ind bf16 {0 @ argmin, -1}
  DVE: per subtile stt (ind+1)*iota_sc, dummy broadcast out,
       accum SUM -> idx_all col = (255-k*)/256
Host: idx = 255-256*accum, s = sqrt(max(mrow+xsq,0)), scatter one-hot rows.
X is host-pre-transposed: loads are plain contiguous DMA (no xbar).
"""

import os
import sys

import numpy as np

sys.path.insert(0, "/opt/trn_rl_repo")

N = 500000
D = 128
K = 256
N_CORES = 8
P = 128
NPC = 62976
N_PAD = N_CORES * NPC
NSUB = NPC // P  # 492
BLOCK_SUBS = [32] * 15 + [12]
PE_VSQ_EVERY = int(os.environ.get("KMEANS_PE_VSQ_EVERY", "1"))  # all chunks: vsq on PE

_nc_cache = {}


def _build():
    from contextlib import ExitStack

    import concourse.bass as bass
    import concourse.tile as tile
    import concourse.tile_sem_assignment as tsa
    from concourse import mybir

    tsa.NUM_HWDGE_SEMS = 1

    f32 = mybir.dt.float32
    bf16 = mybir.dt.bfloat16
    Alu = mybir.AluOpType
    Act = mybir.ActivationFunctionType
    Ax = mybir.AxisListType

    nc = bass.Bass(trn_type="TRN2")
    xhiT_d = nc.dram_tensor("xhiT", [D, NPC], bf16, kind="ExternalInput")
    xloT_d = nc.dram_tensor("xloT", [D, NPC], bf16, kind="ExternalInput")
    whi_d = nc.dram_tensor("whi", [D, K], bf16, kind="ExternalInput")
    wlo_d = nc.dram_tensor("wlo", [D, K], bf16, kind="ExternalInput")
    ones3_d = nc.dram_tensor("ones3", [3, P], bf16, kind="ExternalInput")
    vsq3x4_d = nc.dram_tensor("vsq3x4", [3, 4 * K], bf16, kind="ExternalInput")
    vsqr4_d = nc.dram_tensor("vsqr4", [P, 4 * K], f32, kind="ExternalInput")
    iota_d = nc.dram_tensor("iota_sc", [P, K], bf16, kind="ExternalInput")
    mrow_out_d = nc.dram_tensor("mrow_out", [P, NSUB], f32, kind="ExternalOutput")
    idx_out_d = nc.dram_tensor("idx_out", [P, NSUB], bf16, kind="ExternalOutput")

    def _split_multiwait():
        cnt = 0
        for fn in nc.m.functions:
            for bb in fn.blocks:
                insts = list(bb.instructions)
                out = []
                changed = False
                for ins in insts:
                    si = getattr(ins, "sync_info", None)
                    waits = list(si.on_wait) if (si and si.on_wait) else []
                    if len(waits) > 1:
                        changed = True
                        for wt in waits[:-1]:
                            cnt += 1
                            dr = mybir.InstDrain(name=f"antw-{cnt}", ins=[], outs=[])
                            dr.engine = ins.engine
                            dr.sync_info = mybir.SyncInfo(on_wait=[wt], on_update=[])
                            out.append(dr)
                        ins.sync_info = mybir.SyncInfo(
                            on_wait=[waits[-1]], on_update=list(si.on_update)
                        )
                    out.append(ins)
                if changed:
                    bb.instructions = out
        return cnt

    with tile.TileContext(nc) as tc, ExitStack() as ctx:
        singles = ctx.enter_context(tc.tile_pool(name="singles", bufs=1))
        whi_sb = singles.tile([D, K], bf16)
        nc.sync.dma_start(out=whi_sb, in_=whi_d[:, :])
        wlo_sb = singles.tile([D, K], bf16)
        nc.sync.dma_start(out=wlo_sb, in_=wlo_d[:, :])
        ones3_sb = singles.tile([3, P], bf16)
        nc.sync.dma_start(out=ones3_sb, in_=ones3_d[:, :])
        vsq3x4_sb = singles.tile([3, 4 * K], bf16)
        nc.sync.dma_start(out=vsq3x4_sb, in_=vsq3x4_d[:, :])
        vsqr4_sb = singles.tile([P, 4, K], f32)
        nc.sync.dma_start(out=vsqr4_sb, in_=vsqr4_d[:, :].rearrange("p (g k) -> p g k", g=4))
        iota_sb = singles.tile([P, K], bf16)
        nc.sync.dma_start(out=iota_sb, in_=iota_d[:, :])
        mrow_all = singles.tile([P, NSUB], f32)
        idx_all = singles.tile([P, NSUB], bf16)

        xhip = ctx.enter_context(tc.tile_pool(name="xhiT", bufs=2))
        xlop = ctx.enter_context(tc.tile_pool(name="xloT", bufs=2))
        mps = ctx.enter_context(tc.tile_pool(name="mps", bufs=4, space="PSUM"))
        indp = ctx.enter_context(tc.tile_pool(name="indp", bufs=8))
        dmyp = ctx.enter_context(tc.tile_pool(name="dmyp", bufs=4))

        col0 = 0
        sub0 = 0
        chunk_id = 0
        pending = []

        def _extract(pc, base):
            for g in range(4):
                ind = indp.tile([P, K], bf16, tag="ind")
                nc.scalar.activation(
                    ind,
                    pc[:, g, :],
                    Act.Sign,
                    bias=mrow_all[:, base + g : base + g + 1],
                    scale=-1.0,
                )
                dummy = dmyp.tile([P, 1], bf16, tag="dm")
                nc.vector.scalar_tensor_tensor(
                    out=dummy.broadcast_to((P, K)),
                    in0=ind,
                    scalar=1.0,
                    in1=iota_sb,
                    op0=Alu.add,
                    op1=Alu.mult,
                    accum_out=idx_all[:, base + g : base + g + 1],
                )
        for nsub in BLOCK_SUBS:
            bcols = nsub * P
            xhi_blk = xhip.tile([D, bcols], bf16, tag="xhib")
            nc.sync.dma_start(out=xhi_blk, in_=xhiT_d[:, col0 : col0 + bcols])
            xlo_blk = xlop.tile([D, bcols], bf16, tag="xlob")
            nc.sync.dma_start(out=xlo_blk, in_=xloT_d[:, col0 : col0 + bcols])

            for q in range(0, nsub, 4):
                m_pc = mps.tile([P, 4, K], f32)
                on_pe = (chunk_id % PE_VSQ_EVERY) == 0
                if on_pe:
                    # wide rank-3: one matmul per PSUM bank (512 cols max)
                    for h in range(2):
                        nc.tensor.matmul(
                            m_pc[:, 2 * h : 2 * h + 2, :],
                            lhsT=ones3_sb,
                            rhs=vsq3x4_sb[:, 2 * h * K : (2 * h + 2) * K],
                            start=True,
                            stop=False,
                            skip_group_check=True,
                        )
                else:
                    nc.scalar.copy(out=m_pc[:, :, :], in_=vsqr4_sb)
                for g in range(4):
                    j = q + g
                    sl = m_pc[:, g, :]
                    hi_sl = xhi_blk[:, j * P : (j + 1) * P]
                    lo_sl = xlo_blk[:, j * P : (j + 1) * P]
                    nc.tensor.matmul(
                        sl, lhsT=hi_sl, rhs=whi_sb,
                        start=False, stop=False, skip_group_check=True,
                    )
                    nc.tensor.matmul(
                        sl, lhsT=hi_sl, rhs=wlo_sb,
                        start=False, stop=False, skip_group_check=True,
                    )
                    nc.tensor.matmul(
                        sl, lhsT=lo_sl, rhs=whi_sb,
                        start=False, stop=True, skip_group_check=True,
                    )

                # per-bank reduces: mrow for subtiles 0-1 lands earlier
                nc.vector.tensor_reduce(
                    out=mrow_all[:, sub0 + q : sub0 + q + 2],
                    in_=m_pc[:, 0:2, :],
                    axis=Ax.X,
                    op=Alu.min,
                )
                nc.vector.tensor_reduce(
                    out=mrow_all[:, sub0 + q + 2 : sub0 + q + 4],
                    in_=m_pc[:, 2:4, :],
                    axis=Ax.X,
                    op=Alu.min,
                )

                # software-pipeline: extract indices for the PREVIOUS chunk so
                # DVE/ACT never wait on deps produced this iteration
                pending.append((m_pc, sub0 + q))
                if len(pending) > 1:
                    _extract(*pending.pop(0))
                chunk_id += 1

            col0 += bcols
            sub0 += nsub

        while pending:
            _extract(*pending.pop(0))

        nc.scalar.dma_start(out=mrow_out_d[:, :], in_=mrow_all)
        nc.scalar.dma_start(out=idx_out_d[:, :], in_=idx_all)

    _split_multiwait()
    from concourse.library_overlay import lower_extended_insts

    lower_extended_insts(nc)
    return nc


def _host_prep(X: np.ndarray, V: np.ndarray):
    import ml_dtypes

    bf = ml_dtypes.bfloat16
    V = np.asarray(V, dtype=np.float32)
    wt = np.ascontiguousarray((-2.0 * V).T)  # [D, K] f32

    vsq = np.sum(V * V, axis=1, dtype=np.float32)
    v1 = vsq.astype(bf)
    r = vsq - v1.astype(np.float32)
    v2 = r.astype(bf)
    v3 = (r - v2.astype(np.float32)).astype(bf)
    vsq3 = np.stack([v1, v2, v3])  # [3, K]
    vsq3x4 = np.ascontiguousarray(np.tile(vsq3, (1, 4)))  # [3, 4K]
    ones3 = np.ones((3, P), dtype=bf)
    vsqr4 = np.ascontiguousarray(
        np.broadcast_to(np.tile(vsq, 4)[None, :], (P, 4 * K))
    ).astype(np.float32)
    iota_sc = np.ascontiguousarray(
        np.broadcast_to(
            ((255.0 - np.arange(K, dtype=np.float32)) / 256.0).astype(bf)[None, :],
            (P, K),
        )
    )

    xp = np.zeros((N_PAD, D), dtype=np.float32)
    xp[:N] = X
    xsq = np.einsum("nd,nd->n", xp, xp).astype(np.float32)

    whi = wt.astype(bf)
    wlo = (wt - whi.astype(np.float32)).astype(bf)
    common = dict(
        whi=whi, wlo=wlo, ones3=ones3, vsq3x4=vsq3x4, vsqr4=vsqr4, iota_sc=iota_sc
    )
    per_core = []
    for c in range(N_CORES):
        sl = xp[c * NPC : (c + 1) * NPC]
        hi = sl.astype(bf)
        lo = (sl - hi.astype(np.float32)).astype(bf)
        per_core.append(
            dict(
                xhiT=np.ascontiguousarray(hi.T),
                xloT=np.ascontiguousarray(lo.T),
                **common,
            )
        )
    return per_core, xsq


def kernel(X: np.ndarray, V: np.ndarray) -> np.ndarray:
    from concourse.bass_utils import run_bass_kernel_spmd

    X = np.asarray(X, dtype=np.float32)
    in_maps, xsq = _host_prep(X, V)

    if "h" not in _nc_cache:
        _nc_cache["h"] = _build()
    nc = _nc_cache["h"]

    trace = bool(int(os.environ.get("KMEANS_TRACE", "0")))
    res = run_bass_kernel_spmd(
        nc, in_maps, core_ids=list(range(N_CORES)), trace=trace
    )
    if trace and res.exec_time_ns is not None:
        kernel.last_exec_time_ns = res.exec_time_ns
        kernel.last_mean_exec_time_ns = res.mean_exec_time_ns
        kernel.last_trace = res.instructions_and_trace

    mrow = np.concatenate([r["mrow_out"].T.reshape(-1) for r in res.results])[:N]
    idxf = np.concatenate(
        [r["idx_out"].astype(np.float32).T.reshape(-1) for r in res.results]
    )[:N]
    idx = np.rint(255.0 - 256.0 * idxf).astype(np.int64)
    np.clip(idx, 0, K - 1, out=idx)
    s = np.sqrt(np.maximum(mrow + xsq[:N], 0.0)).astype(np.float32)
    out = np.zeros((N, K), dtype=np.float32)
    out[np.arange(N), idx] = s
    return out


kernel.last_exec_time_ns = None
kernel.last_mean_exec_time_ns = None
kernel.last_trace = None


# revision 6
# speedup vs baseline: 1.2113x; 1.0245x over previous
"""KMeans VQ kernel v3 — bf16 hi/lo GEMM, compact argmin output,
vsq split between PE (wide rank-3 matmul) and ACT (batched PSUM copy).

Per chunk of 4 subtiles (PSUM [P, 4, K] f32):
  vsq init: chunk%4==0 -> PE wide rank-3 matmul (ones3 @ vsq3_x4, 1024 cols)
            else       -> ACT batched copy vsqr4 [P,4,K] f32 SBUF->PSUM
  PE: per subtile 3 bf16 matmuls (hi*whi, hi*wlo, lo*whi) accumulate
  DVE: batched tensor_reduce min -> mrow_all[:, 4 cols]
  ACT: per subtile Sign(mrow - m_s) -> ind bf16 {0 @ argmin, -1}
  DVE: per subtile stt (ind+1)*iota_sc, dummy broadcast out,
       accum SUM -> idx_all col = (255-k*)/256
Host: idx = 255-256*accum, s = sqrt(max(mrow+xsq,0)), scatter one-hot rows.
X is host-pre-transposed: loads are plain contiguous DMA (no xbar).
"""

import os
import sys

import numpy as np

sys.path.insert(0, "/opt/trn_rl_repo")

N = 500000
D = 128
K = 256
N_CORES = 8
P = 128
NPC = 62976
N_PAD = N_CORES * NPC
NSUB = NPC // P  # 492
BLOCK_SUBS = [32] * 15 + [12]
PE_VSQ_EVERY = int(os.environ.get("KMEANS_PE_VSQ_EVERY", "1"))  # all chunks: vsq on PE

_nc_cache = {}


def _build():
    from contextlib import ExitStack

    import concourse.bass as bass
    import concourse.tile as tile
    import concourse.tile_sem_assignment as tsa
    from concourse import mybir

    tsa.NUM_HWDGE_SEMS = 1

    f32 = mybir.dt.float32
    bf16 = mybir.dt.bfloat16
    Alu = mybir.AluOpType
    Act = mybir.ActivationFunctionType
    Ax = mybir.AxisListType

    nc = bass.Bass(trn_type="TRN2")
    xhiT_d = nc.dram_tensor("xhiT", [D, NPC], bf16, kind="ExternalInput")
    xloT_d = nc.dram_tensor("xloT", [D, NPC], bf16, kind="ExternalInput")
    whi_d = nc.dram_tensor("whi", [D, K], bf16, kind="ExternalInput")
    wlo_d = nc.dram_tensor("wlo", [D, K], bf16, kind="ExternalInput")
    ones3_d = nc.dram_tensor("ones3", [3, P], bf16, kind="ExternalInput")
    vsq3x4_d = nc.dram_tensor("vsq3x4", [3, 4 * K], bf16, kind="ExternalInput")
    vsqr4_d = nc.dram_tensor("vsqr4", [P, 4 * K], f32, kind="ExternalInput")
    iota_d = nc.dram_tensor("iota_sc", [P, K], bf16, kind="ExternalInput")
    mrow_out_d = nc.dram_tensor("mrow_out", [P, NSUB], f32, kind="ExternalOutput")
    idx_out_d = nc.dram_tensor("idx_out", [P, NSUB], bf16, kind="ExternalOutput")

    def _split_multiwait():
        cnt = 0
        for fn in nc.m.functions:
            for bb in fn.blocks:
                insts = list(bb.instructions)
                out = []
                changed = False
                for ins in insts:
                    si = getattr(ins, "sync_info", None)
                    waits = list(si.on_wait) if (si and si.on_wait) else []
                    if len(waits) > 1:
                        changed = True
                        for wt in waits[:-1]:
                            cnt += 1
                            dr = mybir.InstDrain(name=f"antw-{cnt}", ins=[], outs=[])
                            dr.engine = ins.engine
                            dr.sync_info = mybir.SyncInfo(on_wait=[wt], on_update=[])
                            out.append(dr)
                        ins.sync_info = mybir.SyncInfo(
                            on_wait=[waits[-1]], on_update=list(si.on_update)
                        )
                    out.append(ins)
                if changed:
                    bb.instructions = out
        return cnt

    with tile.TileContext(nc) as tc, ExitStack() as ctx:
        singles = ctx.enter_context(tc.tile_pool(name="singles", bufs=1))
        whi_sb = singles.tile([D, K], bf16)
        nc.sync.dma_start(out=whi_sb, in_=whi_d[:, :])
        wlo_sb = singles.tile([D, K], bf16)
        nc.sync.dma_start(out=wlo_sb, in_=wlo_d[:, :])
        ones3_sb = singles.tile([3, P], bf16)
        nc.sync.dma_start(out=ones3_sb, in_=ones3_d[:, :])
        vsq3x4_sb = singles.tile([3, 4 * K], bf16)
        nc.sync.dma_start(out=vsq3x4_sb, in_=vsq3x4_d[:, :])
        vsqr4_sb = singles.tile([P, 4, K], f32)
        nc.sync.dma_start(out=vsqr4_sb, in_=vsqr4_d[:, :].rearrange("p (g k) -> p g k", g=4))
        iota_sb = singles.tile([P, K], bf16)
        nc.sync.dma_start(out=iota_sb, in_=iota_d[:, :])
        mrow_all = singles.tile([P, NSUB], f32)
        idx_all = singles.tile([P, NSUB], bf16)

        xhip = ctx.enter_context(tc.tile_pool(name="xhiT", bufs=2))
        xlop = ctx.enter_context(tc.tile_pool(name="xloT", bufs=2))
        mps = ctx.enter_context(tc.tile_pool(name="mps", bufs=4, space="PSUM"))
        indp = ctx.enter_context(tc.tile_pool(name="indp", bufs=8))
        dmyp = ctx.enter_context(tc.tile_pool(name="dmyp", bufs=4))

        col0 = 0
        sub0 = 0
        chunk_id = 0
        pending = []

        def _extract(pc, base):
            for g in range(4):
                ind = indp.tile([P, K], bf16, tag="ind")
                nc.scalar.activation(
                    ind,
                    pc[:, g, :],
                    Act.Sign,
                    bias=mrow_all[:, base + g : base + g + 1],
                    scale=-1.0,
                )
                dummy = dmyp.tile([P, 1], bf16, tag="dm")
                nc.vector.scalar_tensor_tensor(
                    out=dummy.broadcast_to((P, K)),
                    in0=ind,
                    scalar=1.0,
                    in1=iota_sb,
                    op0=Alu.add,
                    op1=Alu.mult,
                    accum_out=idx_all[:, base + g : base + g + 1],
                )
        for nsub in BLOCK_SUBS:
            bcols = nsub * P
            xhi_blk = xhip.tile([D, bcols], bf16, tag="xhib")
            nc.sync.dma_start(out=xhi_blk, in_=xhiT_d[:, col0 : col0 + bcols])
            xlo_blk = xlop.tile([D, bcols], bf16, tag="xlob")
            nc.sync.dma_start(out=xlo_blk, in_=xloT_d[:, col0 : col0 + bcols])

            for q in range(0, nsub, 4):
                m_pc = mps.tile([P, 4, K], f32)
                on_pe = (chunk_id % PE_VSQ_EVERY) == 0
                if on_pe:
                    # wide rank-3: one matmul per PSUM bank (512 cols max)
                    for h in range(2):
                        nc.tensor.matmul(
                            m_pc[:, 2 * h : 2 * h + 2, :],
                            lhsT=ones3_sb,
                            rhs=vsq3x4_sb[:, 2 * h * K : (2 * h + 2) * K],
                            start=True,
                            stop=False,
                            skip_group_check=True,
                        )
                else:
                    nc.scalar.copy(out=m_pc[:, :, :], in_=vsqr4_sb)
                for g in range(4):
                    j = q + g
                    sl = m_pc[:, g, :]
                    hi_sl = xhi_blk[:, j * P : (j + 1) * P]
                    lo_sl = xlo_blk[:, j * P : (j + 1) * P]
                    nc.tensor.matmul(
                        sl, lhsT=hi_sl, rhs=whi_sb,
                        start=False, stop=False, skip_group_check=True,
                    )
                    nc.tensor.matmul(
                        sl, lhsT=hi_sl, rhs=wlo_sb,
                        start=False, stop=False, skip_group_check=True,
                    )
                    nc.tensor.matmul(
                        sl, lhsT=lo_sl, rhs=whi_sb,
                        start=False, stop=True, skip_group_check=True,
                    )

                # per-subtile reduces: each mrow lands as soon as its
                # subtile's matmuls finish
                for g in range(4):
                    nc.vector.tensor_reduce(
                        out=mrow_all[:, sub0 + q + g : sub0 + q + g + 1],
                        in_=m_pc[:, g, :],
                        axis=Ax.X,
                        op=Alu.min,
                    )

                # software-pipeline: extract indices for the PREVIOUS chunk so
                # DVE/ACT never wait on deps produced this iteration
                pending.append((m_pc, sub0 + q))
                if len(pending) > 1:
                    _extract(*pending.pop(0))
                chunk_id += 1

            col0 += bcols
            sub0 += nsub

        while pending:
            _extract(*pending.pop(0))

        nc.scalar.dma_start(out=mrow_out_d[:, :], in_=mrow_all)
        nc.scalar.dma_start(out=idx_out_d[:, :], in_=idx_all)

    _split_multiwait()
    from concourse.library_overlay import lower_extended_insts

    lower_extended_insts(nc)
    return nc


def _host_prep(X: np.ndarray, V: np.ndarray):
    import ml_dtypes

    bf = ml_dtypes.bfloat16
    V = np.asarray(V, dtype=np.float32)
    wt = np.ascontiguousarray((-2.0 * V).T)  # [D, K] f32

    vsq = np.sum(V * V, axis=1, dtype=np.float32)
    v1 = vsq.astype(bf)
    r = vsq - v1.astype(np.float32)
    v2 = r.astype(bf)
    v3 = (r - v2.astype(np.float32)).astype(bf)
    vsq3 = np.stack([v1, v2, v3])  # [3, K]
    vsq3x4 = np.ascontiguousarray(np.tile(vsq3, (1, 4)))  # [3, 4K]
    ones3 = np.ones((3, P), dtype=bf)
    vsqr4 = np.ascontiguousarray(
        np.broadcast_to(np.tile(vsq, 4)[None, :], (P, 4 * K))
    ).astype(np.float32)
    iota_sc = np.ascontiguousarray(
        np.broadcast_to(
            ((255.0 - np.arange(K, dtype=np.float32)) / 256.0).astype(bf)[None, :],
            (P, K),
        )
    )

    xp = np.zeros((N_PAD, D), dtype=np.float32)
    xp[:N] = X
    xsq = np.einsum("nd,nd->n", xp, xp).astype(np.float32)

    whi = wt.astype(bf)
    wlo = (wt - whi.astype(np.float32)).astype(bf)
    common = dict(
        whi=whi, wlo=wlo, ones3=ones3, vsq3x4=vsq3x4, vsqr4=vsqr4, iota_sc=iota_sc
    )
    per_core = []
    for c in range(N_CORES):
        sl = xp[c * NPC : (c + 1) * NPC]
        hi = sl.astype(bf)
        lo = (sl - hi.astype(np.float32)).astype(bf)
        per_core.append(
            dict(
                xhiT=np.ascontiguousarray(hi.T),
                xloT=np.ascontiguousarray(lo.T),
                **common,
            )
        )
    return per_core, xsq


def kernel(X: np.ndarray, V: np.ndarray) -> np.ndarray:
    from concourse.bass_utils import run_bass_kernel_spmd

    X = np.asarray(X, dtype=np.float32)
    in_maps, xsq = _host_prep(X, V)

    if "h" not in _nc_cache:
        _nc_cache["h"] = _build()
    nc = _nc_cache["h"]

    trace = bool(int(os.environ.get("KMEANS_TRACE", "0")))
    res = run_bass_kernel_spmd(
        nc, in_maps, core_ids=list(range(N_CORES)), trace=trace
    )
    if trace and res.exec_time_ns is not None:
        kernel.last_exec_time_ns = res.exec_time_ns
        kernel.last_mean_exec_time_ns = res.mean_exec_time_ns
        kernel.last_trace = res.instructions_and_trace

    mrow = np.concatenate([r["mrow_out"].T.reshape(-1) for r in res.results])[:N]
    idxf = np.concatenate(
        [r["idx_out"].astype(np.float32).T.reshape(-1) for r in res.results]
    )[:N]
    idx = np.rint(255.0 - 256.0 * idxf).astype(np.int64)
    np.clip(idx, 0, K - 1, out=idx)
    s = np.sqrt(np.maximum(mrow + xsq[:N], 0.0)).astype(np.float32)
    out = np.zeros((N, K), dtype=np.float32)
    out[np.arange(N), idx] = s
    return out


kernel.last_exec_time_ns = None
kernel.last_mean_exec_time_ns = None
kernel.last_trace = None
